# revision 1
# baseline (speedup 1.0000x reference)
"""VQ codebook lookup (CVQVAE) for Trainium2, data-parallel over 8 NeuronCores.

Math per token row x (D=64), codebook e (K=2048, D=64):
    d_j = ||x||^2 + ||e_j||^2 - 2 x.e_j ;  idx = argmin_j d_j ;  z_q = e[idx]

To reproduce the reference's fp32 argmin bit-for-bit (including tie behavior),
we compute on-device the negated score
    s_j = fl(fl(2x.e_j - x_sq) - e_sq_j)
which equals -fl(fl(x_sq - 2x.e_j) + e_sq_j) exactly; its argmax with
first-index ties equals the reference argmin (verified numerically: the
e_sq contribution must be rounded at the x_sq magnitude in a separate step,
but the exact association order / matmul accumulation order do not matter).

Per 128-token tile:
  PE   : m2 = x @ (2*codebook).T            -> PSUM [128, 2048]
  ACT  : x_sq via Square+accum              -> [128, 1]
  DVE  : s = (m2 - x_sq) - e_sq_bcast       (scalar_tensor_tensor, one pass)
  GPS  : M = rowmax(s)
  DVE  : max_index(s, M) -> first argmax index (uint32)
  GPS  : indirect DMA gather codebook[idx]  -> z_q tile
"""

import numpy as np

P = 128          # partitions / tokens per tile
K = 2048         # codebook entries
D = 64           # latent dim
N_CORES = 8
N_FULL = 131072
N_LOC = N_FULL // N_CORES   # 16384 tokens per core


def build_nc(n_loc=N_LOC):
    import concourse.bass as bass
    import concourse.tile as tile
    from concourse import bacc, mybir

    f32 = mybir.dt.float32
    u32 = mybir.dt.uint32
    Alu = mybir.AluOpType
    Act = mybir.ActivationFunctionType

    n_tiles = n_loc // P

    nc = bacc.Bacc("TRN2", target_bir_lowering=False, debug=False)

    x_d = nc.dram_tensor("x", [n_loc, D], f32, kind="ExternalInput").ap()
    cb_d = nc.dram_tensor("codebook", [K, D], f32, kind="ExternalInput").ap()
    cbT_d = nc.dram_tensor("codebookT", [D, K], f32, kind="ExternalInput").ap()
    zq_d = nc.dram_tensor("z_q", [n_loc, D], f32, kind="ExternalOutput").ap()

    with tile.TileContext(nc) as tc:
        with (
            tc.tile_pool(name="const", bufs=1) as cpool,
            tc.tile_pool(name="xin", bufs=4) as xpool,
            tc.tile_pool(name="xt", bufs=4) as xtpool,
            tc.tile_pool(name="score", bufs=3) as spool,
            tc.tile_pool(name="small", bufs=4) as smpool,
            tc.tile_pool(name="zq", bufs=4) as zqpool,
        ):
            # ---------------- one-time setup ----------------
            cbT = cpool.tile([D, K], f32)           # codebook.T
            nc.sync.dma_start(cbT[:], cbT_d[:, :])

            cb2T = cpool.tile([D, K], f32)          # 2 * codebook.T (matmul rhs)
            nc.scalar.activation(cb2T[:], cbT[:], Act.Copy, scale=2.0)

            cbsq = cpool.tile([D, K], f32)          # (2e)^2
            nc.scalar.activation(cbsq[:], cb2T[:], Act.Square)

            ones = cpool.tile([D, 1], f32)
            nc.vector.memset(ones[:], 1.0)

            # e_sq row: colsum((2e)^2) * 0.25 == fl(sum e^2) exactly
            esq_row = cpool.tile([1, K], f32)
            with tc.tile_pool(name="setup_ps", bufs=1, space="PSUM") as spsum:
                for c in range(K // 512):
                    ps = spsum.tile([1, 512], f32, tag="esq_ps")
                    nc.tensor.matmul(ps[:], lhsT=ones[:],
                                     rhs=cbsq[:, c * 512:(c + 1) * 512],
                                     start=True, stop=True)
                    nc.scalar.activation(esq_row[:, c * 512:(c + 1) * 512], ps[:],
                                         Act.Copy, scale=0.25)

            # broadcast e_sq row to all 128 partitions via a DRAM bounce
            # (DRAM tensors must be 2-D for the runtime loader)
            esq_stage = nc.dram_tensor("esq_stage", [1, K], f32,
                                       kind="ExternalOutput").ap()
            nc.sync.dma_start(esq_stage[:, :], esq_row[0:1, :])
            esq_b = cpool.tile([P, K], f32)
            nc.sync.dma_start(esq_b[:], esq_stage[0, :].partition_broadcast(P))

            # persistent in_max for max_index: col 0 gets each tile's row max;
            # cols 1-7 are never read meaningfully (max_index first-match
            # semantics only needs col 0) but must be initialized.
            mx8 = cpool.tile([P, 8], f32)
            nc.gpsimd.memset(mx8[:], 0.0)

            # ---------------- main loop ----------------
            mpsum_cm = tc.tile_pool(name="mm", bufs=2, space="PSUM")
            mpsum = mpsum_cm.__enter__()
            for ti in range(n_tiles):
                t0 = ti * P

                x_t = xpool.tile([P, D], f32)
                nc.sync.dma_start(x_t[:], x_d[t0:t0 + P, :])

                xT_t = xtpool.tile([D, P], f32)
                with nc.allow_non_contiguous_dma(reason="transposed x load"):
                    nc.sync.dma_start(xT_t[:], x_d[t0:t0 + P, :].rearrange("t d -> d t"))

                # x_sq via Square + free-dim accumulate
                sq_t = xpool.tile([P, D], f32, tag="sqjunk")
                x_sq = smpool.tile([P, 1], f32, tag="xsq")
                nc.scalar.activation(sq_t[:], x_t[:], Act.Square, accum_out=x_sq[:])

                # m2 = x @ (2 cb)^T  -> [128, 2048] fp32 PSUM (4 banks)
                m2 = mpsum.tile([P, K], f32)
                for q in range(K // 512):
                    nc.tensor.matmul(m2[:, q * 512:(q + 1) * 512], lhsT=xT_t[:],
                                     rhs=cb2T[:, q * 512:(q + 1) * 512],
                                     start=True, stop=True)

                # s = (m2 - x_sq) - e_sq   (one DVE pass, ref-exact rounding)
                s_t = spool.tile([P, K], f32)
                nc.vector.scalar_tensor_tensor(
                    out=s_t[:], in0=m2[:], scalar=x_sq[:], in1=esq_b[:],
                    op0=Alu.subtract, op1=Alu.subtract)

                # row max: DVE tensor_scalar bypass + max-accum (2x_2p mode),
                # written straight into in_max col 0 (no ACT broadcast hop)
                s_junk = spool.tile([P, K], f32, tag="sjunk")
                nc.vector.tensor_scalar(s_junk[:], s_t[:], scalar1=0.0, scalar2=None,
                                        op0=Alu.bypass, op1=Alu.max,
                                        accum_out=mx8[:, 0:1])

                # first index of the max (reference argmin tie semantics)
                idx8 = smpool.tile([P, 8], u32, tag="idx8")
                nc.vector.max_index(idx8[:], mx8[:], s_t[:])

                # gather codebook rows by index (DRAM -> SBUF), then store
                zq_t = zqpool.tile([P, D], f32)
                nc.gpsimd.indirect_dma_start(
                    out=zq_t[:], out_offset=None, in_=cb_d[:, :],
                    in_offset=bass.IndirectOffsetOnAxis(ap=idx8[:, 0:1], axis=0))
                nc.sync.dma_start(zq_d[t0:t0 + P, :], zq_t[:])
            mpsum_cm.__exit__(None, None, None)

    nc.compile()
    return nc


_NC_CACHE = {}


def _get_nc(n_loc):
    if n_loc not in _NC_CACHE:
        _NC_CACHE[n_loc] = build_nc(n_loc)
    return _NC_CACHE[n_loc]


def kernel(x: np.ndarray, codebook: np.ndarray) -> np.ndarray:
    from concourse import bass_utils

    x = np.ascontiguousarray(np.asarray(x, dtype=np.float32))
    cb = np.ascontiguousarray(np.asarray(codebook, dtype=np.float32))
    n = x.shape[0]
    n_loc = n // N_CORES
    cbT = np.ascontiguousarray(cb.T)

    nc = _get_nc(n_loc)
    in_maps = [
        {"x": x[i * n_loc:(i + 1) * n_loc], "codebook": cb, "codebookT": cbT}
        for i in range(N_CORES)
    ]
    res = bass_utils.run_bass_kernel_spmd(nc, in_maps, list(range(N_CORES))).results
    return np.concatenate([res[i]["z_q"] for i in range(N_CORES)], axis=0)

